# revision 1
# baseline (speedup 1.0000x reference)
"""Trainium2 Bass kernel for nn_AttentionBlock (GroupNorm + single-head spatial
self-attention + residual) on 8 NeuronCores.

Sharding: data-parallel over batch (2) x sequence-parallel over the query
dimension (4 chunks of 1024 of the 4096 spatial tokens). Each core gets the
full image of its batch element, ROTATED so its query chunk sits at token 0
(GroupNorm stats, key/value sets and softmax sums are permutation-invariant
over tokens, so rotation lets all 8 cores run the identical SPMD program).

All matmul operands are bf16 (measured ~7% faster streaming than f32r, and
half the SBUF/DMA traffic). x ships as bf16 and stays RESIDENT in SBUF for
the whole kernel: GroupNorm stats, all projections, and the residual read it
in place -- no second pass over HBM.

Per-core dataflow (channel-major [C on partitions] everywhere except v):
  phase 1: x DMA (bf16, 4MB, 16 quarter-tile transfers) with bn_stats per
           512-token chunk streaming behind the DMAs on DVE; group-combine
           via a tiny PE matmul with a 1/16 block-indicator, broadcast back
           with a second indicator matmul -> per-channel Scale a[c] /
           Bias b[c].
  fold:    the GroupNorm affine never touches x: the scale a is folded into
           the wq/wk/wv weight tiles (one in-place per-partition multiply on
           each [128, 512] weight tile), and the shift b becomes per-output-
           channel constants computed as ROWS (b^T . wkT = (wk.b)^T, four
           N=512 matmuls per projection) then moved to partition vectors
           with tiny K=1 transpose matmuls; v's constant rides through
           softmax (attention rows sum to 1) and lands in the y epilogue
           constant yb = wp.(wv.b+bv)+bp.
  phase 2: project q/k/v straight out of the resident x tiles:
           k [C, 4096], vT [4096, C] (transposed layout so the later AV
           matmul needs no transposes), q [C, 1024].
  phase 3: per 512-query half: scores^T [j:128, i:512] = k_tile^T @ q (PSUM
           accum over C), exp on ScalarE straight out of PSUM (no max
           subtraction -- logits are O(5)), row-sums r via a ones-vector
           matmul, AV accum hattn[c, i] += vT_tile^T @ p with no transposes.
           Softmax normalization is deferred PAST the output projection
           ((wp.po)/r == wp.(po/r)): unnormalized po is copied to bf16, the
           wp matmuls run immediately, and the 1/r reciprocal + broadcast
           runs in parallel on DVE; the final y = py*rb + (x + yb) is two
           DVE ops against a precomputed x+yb tile. The next half's first
           score/exp iterations are emitted into the tail window so the PE
           never idles across halves.
"""

import sys
from contextlib import ExitStack

if "/opt/trn_rl_repo" not in sys.path:
    sys.path.insert(0, "/opt/trn_rl_repo")

import numpy as np

import concourse.bass as bass  # noqa: F401  (import keeps bass registered)
import concourse.tile as tile
from concourse import bacc, mybir
from concourse.alu_op_type import AluOpType
from concourse.bass_utils import run_bass_kernel_spmd

F32 = mybir.dt.float32
BF16 = mybir.dt.bfloat16
AF = mybir.ActivationFunctionType
OP = AluOpType

B, C, H, W = 2, 512, 64, 64
HW = H * W          # 4096 spatial tokens
P = 128             # partitions
CT = C // P         # 4 channel tiles
NCORES = 8
QN = HW // 4        # 1024 queries per core
CHW = 512           # token chunk width
NCH = HW // CHW     # 8 chunks
JT = HW // P        # 32 key tiles
EPS = 1e-6
SCALE = float(C) ** -0.5
GPT = P // 16       # 8 groups per channel tile
NCHA = 5            # chunks 0..4 on the DVE bn_stats path
NA = NCHA * CHW     # 2560 tokens (path A)
NB = HW - NA        # 1536 tokens (path B: ScalarE sumsq + PE sums)


def _build_body(nc, tc, ctx, d):
    xb_d = d["xb"]
    wT_d = {n: d[n] for n in ("wqT", "wkT", "wvT", "wpT")}
    y_d = d["y"]

    cpool = ctx.enter_context(tc.tile_pool(name="const", bufs=1))
    ppool = ctx.enter_context(tc.tile_pool(name="persist", bufs=1))
    spool = ctx.enter_context(tc.tile_pool(name="stream", bufs=2))
    smpool = ctx.enter_context(tc.tile_pool(name="small", bufs=1))
    qpool = ctx.enter_context(tc.tile_pool(name="psum", bufs=3, space="PSUM"))

    dma_engines = [nc.gpsimd, nc.scalar, nc.sync]

    # ---- phase 1: x arrives bf16 (quarter-tile transfers, earliest tokens
    # first) and stays resident for the whole kernel ----
    x_sb = [ppool.tile([P, HW], BF16, tag=f"x{t}", name=f"x{t}")
            for t in range(CT)]
    for t in range(CT):
        for qtr in range(4):
            eng = dma_engines[(t * 4 + qtr) % 3]
            if t == 0 and qtr == 0:
                # split the very first transfer so the first bn_stats chunk
                # lands (and the DVE-serial stats stream starts) earlier
                eng.dma_start(x_sb[0][:, 0:CHW], xb_d[0, 0, :, 0:CHW])
                eng.dma_start(x_sb[0][:, CHW:QN], xb_d[0, 0, :, CHW:QN])
            else:
                eng.dma_start(x_sb[t][:, qtr * QN:(qtr + 1) * QN],
                              xb_d[t, qtr])
    # small constants right behind the x stream
    ind = cpool.tile([P, GPT], F32, tag="ind")
    nc.gpsimd.dma_start(ind[:], d["ind"][:])
    indb = cpool.tile([P, GPT], BF16, tag="indb")
    nc.scalar.dma_start(indb[:], d["indb"][:])
    indT = cpool.tile([GPT, P], F32, tag="indT")
    nc.sync.dma_start(indT[:], d["indT"][:])
    chv24 = cpool.tile([P, 6 * CT], F32, tag="chv24")
    nc.gpsimd.dma_start(chv24[:], d["chv"][:])
    # chv columns per tile t at 6*t+j: gamma, beta, bq, bk, bv, bp
    chv3 = chv24.rearrange("p (t six) -> p t six", six=6)

    def chvcol(t, c):
        return chv24[:, 6 * t + c:6 * t + c + 1]
    # bulk weights, in consumption order, spread across the 3 DMA queues
    wts = {}
    for wi, name in enumerate(("wkT", "wvT", "wqT", "wpT")):
        wts[name] = []
        for t in range(CT):
            w = cpool.tile([P, C], BF16, tag=f"{name}{t}", name=f"{name}{t}")
            dma_engines[(wi * CT + t) % 3].dma_start(w[:], wT_d[name][t])
            wts[name].append(w)

    ones_col = cpool.tile([P, 1], BF16, tag="onesc")
    nc.vector.memset(ones_col[:], 1.0)
    ones_row = smpool.tile([1, P], BF16, tag="onesr")
    nc.vector.memset(ones_row[:], 1.0)
    one_one = smpool.tile([1, 2], F32, tag="one1")
    nc.vector.memset(one_one[:], 1.0)
    epst = smpool.tile([GPT, CT], F32, tag="eps")
    nc.vector.memset(epst[:], EPS)
    # preload the Sqrt activation table while ScalarE is idle, so the
    # GroupNorm rstd Sqrt doesn't eat a 1.3us just-in-time table load on
    # the critical aggregation chain
    dumt = smpool.tile([GPT, 1], F32, tag="dumt")
    nc.scalar.activation(dumt[:], epst[:, 0:1], AF.Sqrt)

    # GroupNorm stats: DVE bn_stats, tile-major so each tile's aggregation
    # and group-combine matmul overlap the next tile's stats
    sts = [smpool.tile([P, NCH, 6], F32, tag="st", bufs=CT, name=f"st{t}")
           for t in range(CT)]
    gps = qpool.tile([GPT, 2 * CT], F32, tag="pa")
    for t in range(CT):
        for ch in range(NCH):
            nc.vector.bn_stats(sts[t][:, ch, :],
                               x_sb[t][:, ch * CHW:(ch + 1) * CHW])
        mv = smpool.tile([P, 2], F32, tag="mv", bufs=2)
        nc.vector.bn_aggr(mv[:], sts[t][:])
        s3 = smpool.tile([P, 2], F32, tag="s3", bufs=2)
        nc.vector.tensor_copy(s3[:, 0:1], mv[:, 0:1])
        sq = smpool.tile([P, 1], F32, tag="sq", bufs=2)
        nc.vector.tensor_tensor(sq[:], mv[:, 0:1], mv[:, 0:1], op=OP.mult)
        nc.vector.tensor_tensor(s3[:, 1:2], sq[:], mv[:, 1:2], op=OP.add)
        nc.tensor.matmul(gps[:, 2 * t:2 * t + 2], ind[:], s3[:],
                         start=True, stop=True)

    gst = smpool.tile([GPT, 2 * CT], F32, tag="gst")
    nc.vector.tensor_copy(gst[:], gps[:])
    g3 = gst.rearrange("p (t two) -> p t two", two=2)
    mu = smpool.tile([GPT, CT], F32, tag="mu")
    nc.vector.tensor_copy(mu[:], g3[:, :, 0])
    msq = smpool.tile([GPT, CT], F32, tag="msq")
    nc.vector.tensor_tensor(msq[:], mu[:], mu[:], op=OP.mult)
    varg = smpool.tile([GPT, CT], F32, tag="varg")
    nc.vector.tensor_tensor(varg[:], g3[:, :, 1], msq[:], op=OP.subtract)
    stdg = smpool.tile([GPT, CT], F32, tag="stdg")
    nc.scalar.activation(stdg[:], varg[:], AF.Sqrt, bias=epst[:, 0:1])
    # preload the Exp table right after the last Sqrt (v-copies in between
    # are table-neutral), so phase 3's first exp starts without a reload
    nc.scalar.activation(dumt[:], stdg[:, 0:1], AF.Exp)
    # interleave (mu_t, rstd_t) columns and broadcast all groups->channels
    # with a single [K=8, M=128, N=8] indicator matmul
    mr = smpool.tile([GPT, 2 * CT], F32, tag="mr")
    mr3 = mr.rearrange("p (t two) -> p t two", two=2)
    nc.vector.tensor_copy(mr3[:, :, 0], mu[:])
    nc.vector.reciprocal(mr3[:, :, 1], stdg[:])
    cba = qpool.tile([P, 2 * CT], F32, tag="pa")
    nc.tensor.matmul(cba[:], indT[:], mr[:], start=True, stop=True)
    cb = smpool.tile([P, 2 * CT], F32, tag="cb")
    nc.vector.tensor_copy(cb[:], cba[:])

    # per-channel Scale a / Bias b, vectorized across all 4 tiles via
    # strided views (one wide DVE op instead of one per tile)
    cb2 = cb.rearrange("p (t two) -> p t two", two=2)
    ab = ppool.tile([P, 2, CT], F32, tag="ab")   # [:,0,t]=a_t  [:,1,t]=b_t
    nc.vector.tensor_tensor(ab[:, 0, :], cb2[:, :, 1], chv3[:, :, 0],
                            op=OP.mult)
    tmpb = smpool.tile([P, CT], F32, tag="tmpb", bufs=1)
    nc.vector.tensor_tensor(tmpb[:], cb2[:, :, 0], ab[:, 0, :], op=OP.mult)
    nc.vector.tensor_tensor(ab[:, 1, :], chv3[:, :, 1], tmpb[:],
                            op=OP.subtract)
    bvec_all = ppool.tile([P, CT, 2], BF16, tag="bva")
    nc.vector.tensor_copy(bvec_all[:, :, 0], ab[:, 1, :])
    nc.vector.tensor_copy(bvec_all[:, :, 1], ab[:, 1, :])
    sbts = [ab[:, 0, t:t + 1] for t in range(CT)]
    bvec = [bvec_all[:, t, :] for t in range(CT)]

    # GroupNorm scale folded into SCALED COPIES of the projection weights
    # (the raw tiles stay live for the bias contracts interleaved into
    # phase 2 below)
    ws = {"wpT": wts["wpT"]}
    for name in ("wkT", "wvT", "wqT"):
        ws[name] = []
        for t in range(CT):
            w = cpool.tile([P, C], BF16, tag=f"s{name}{t}", name=f"s{name}{t}")
            nc.vector.tensor_scalar_mul(w[:], wts[name][t][:],
                                        sbts[t])
            ws[name].append(w)

    # ---- bias-term constants from RAW weights (tiny N=2 matmuls);
    # emitted one output-tile group at a time, interleaved into phase 2's
    # first chunks so they never head-of-line block the projections ----
    #   qb[o] = sum_c wq[o,c] b[c] + bq    (added at the q PSUM->SBUF move)
    #   kb[o] = likewise with bk
    #   vbt[c] = sum_cin wv[c,cin] b[cin] + bv   (rides softmax into yb)
    #   yb[o] = sum_c wp[o,c] vbt[c] + bp        (y epilogue constant)
    def bias_ct(wname, ot, rhs_tiles, outdt, addcol, tagp):
        pb = qpool.tile([P, 2], F32, tag="pa", name="pb")
        for t in range(CT):
            nc.tensor.matmul(pb[:], wts[wname][t][:, ot * P:(ot + 1) * P],
                             rhs_tiles[t][:, 0:2], start=(t == 0),
                             stop=(t == CT - 1))
        w = 2 if outdt == BF16 else 1
        ob = ppool.tile([P, w], outdt, tag=f"{tagp}{ot}", name=f"{tagp}{ot}")
        if outdt == F32:
            nc.vector.tensor_scalar(ob[:], pb[:, 0:1],
                                    chvcol(ot, addcol),
                                    None, OP.add)
        else:
            tf = smpool.tile([P, 1], F32, tag="tf", bufs=2)
            nc.vector.tensor_scalar(tf[:], pb[:, 0:1],
                                    chvcol(ot, addcol),
                                    None, OP.add)
            nc.vector.tensor_copy(ob[:, 0:1], tf[:])
            nc.vector.tensor_copy(ob[:, 1:2], tf[:])
        return ob

    # ---- persistent attention operands (all bf16) ----
    k_sb = [ppool.tile([P, HW], BF16, tag=f"k{t}", name=f"k{t}")
            for t in range(CT)]
    q_sb = [ppool.tile([P, QN], BF16, tag=f"q{t}", name=f"q{t}")
            for t in range(CT)]
    vT_sb = [ppool.tile([P, C], BF16, tag=f"vT{j}", name=f"vT{j}")
             for j in range(JT)]

    # ---- phase 2: q/k/v projections straight from resident x; the bias
    # contracts ride along inside chunks 0-1 (their DVE adds wait, the
    # projection matmuls never do) ----
    kb, qb, vbt, yb = [None] * CT, [None] * CT, [None] * CT, [None] * CT
    # phase-2 projection chains rotate across ALL 8 PSUM banks (the po
    # attention accumulators are idle until phase 3), so the PE never waits
    # on a PSUM slot still being drained by a DVE/ScalarE consumer
    p2tags = ["pa", "po0", "po1", "po2", "po3"]
    p2cnt = [0]

    def p2psum():
        tag = p2tags[p2cnt[0] % len(p2tags)]
        p2cnt[0] += 1
        return qpool.tile([P, CHW], F32, tag=tag,
                          bufs=(3 if tag == "pa" else 1), name="p2")

    for ch in range(NCH):
        sl = slice(ch * CHW, (ch + 1) * CHW)
        for ot in range(CT):
            pk = p2psum()
            for t in range(CT):
                nc.tensor.matmul(pk[:], ws["wkT"][t][:, ot * P:(ot + 1) * P],
                                 x_sb[t][:, sl], start=(t == 0),
                                 stop=(t == CT - 1))
            if ch == 0:
                kb[ot] = bias_ct("wkT", ot, bvec, F32, 3, "kb")
            nc.vector.tensor_scalar(k_sb[ot][:, sl], pk[:], kb[ot][:, 0:1],
                                    None, OP.add)
        for nt in range(CT):
            pv = p2psum()
            for t in range(CT):
                nc.tensor.matmul(pv[:], x_sb[t][:, ch * CHW + nt * P:
                                                 ch * CHW + (nt + 1) * P],
                                 ws["wvT"][t][:], start=(t == 0),
                                 stop=(t == CT - 1))
            if ch == 0:
                vbt[nt] = bias_ct("wvT", nt, bvec, BF16, 4, "vbt")
            elif ch == 1:
                yb[nt] = bias_ct("wpT", nt, vbt, F32, 5, "yb")
            nc.scalar.copy(vT_sb[ch * CT + nt][:], pv[:])
        if ch * CHW < QN:
            for ot in range(CT):
                pq = p2psum()
                for t in range(CT):
                    nc.tensor.matmul(pq[:],
                                     ws["wqT"][t][:, ot * P:(ot + 1) * P],
                                     x_sb[t][:, sl], start=(t == 0),
                                     stop=(t == CT - 1))
                if ch == 0:
                    qb[ot] = bias_ct("wqT", ot, bvec, F32, 2, "qb")
                nc.vector.tensor_scalar(q_sb[ot][:, sl], pq[:], qb[ot][:, 0:1],
                                        None, OP.add)

    # x + yb, precomputed off the critical path for the y epilogue
    xyb = [[None] * CT for _ in range(2)]
    for ih in range(2):
        for ot in range(CT):
            xt = ppool.tile([P, CHW], F32, tag=f"xyb{ih}{ot}",
                            name=f"xyb{ih}{ot}")
            nc.vector.tensor_scalar(xt[:],
                                    x_sb[ot][:, ih * CHW:(ih + 1) * CHW],
                                    yb[ot][:, 0:1], None, OP.add)
            xyb[ih][ot] = xt

    # ---- phase 3: attention, per query half ----
    def mk_pr():
        return qpool.tile([1, CHW], F32, tag="pr", bufs=1, name="pr")

    def mk_po():
        return [qpool.tile([P, CHW], F32, tag=f"po{t}", name=f"po{t}", bufs=1)
                for t in range(CT)]

    def sc_exp(ih, j):
        isl = slice(ih * CHW, (ih + 1) * CHW)
        ps_ = qpool.tile([P, CHW], F32, tag="pa", name="ps")
        for t in range(CT):
            nc.tensor.matmul(ps_[:], k_sb[t][:, j * P:(j + 1) * P],
                             q_sb[t][:, isl], start=(t == 0),
                             stop=(t == CT - 1))
        pT = spool.tile([P, CHW], BF16, tag="pT", bufs=24, name="pT")
        nc.scalar.activation(pT[:], ps_[:], AF.Exp, scale=SCALE)
        return pT

    def av_only(po, j, pT):
        for t in range(CT):
            nc.tensor.matmul(po[t][:], vT_sb[j][:, t * P:(t + 1) * P],
                             pT[:], start=(j == 0), stop=(j == JT - 1))

    def rowsum_only(pr, j, pT):
        nc.tensor.matmul(pr[:], ones_col[:], pT[:],
                         start=(j == 0), stop=(j == JT - 1))

    def tail_and_y(pr, po, ih):
        isl = slice(ih * CHW, (ih + 1) * CHW)
        # 1/r chain first: pr closed early (rowsum bursts), so DVE computes
        # rinv/rbb BEFORE the last AV finishes and the broadcast matmul
        # issues at tail start instead of after the first wp chain
        rinv = smpool.tile([1, CHW], F32, tag="rinv", bufs=2)
        nc.vector.reciprocal_approx_fast(rinv[:], pr[:])
        rbb = smpool.tile([1, CHW], BF16, tag="rbb", bufs=2)
        nc.vector.tensor_copy(rbb[:], rinv[:])
        rb = spool.tile([P, CHW], F32, tag="rb", bufs=2)
        prb = qpool.tile([P, CHW], F32, tag="pa")
        nc.tensor.matmul(prb[:], ones_row[:], rbb[:], start=True, stop=True)
        # unnormalized h -> bf16 so the wp matmuls start immediately
        # ((wp.po)/r == wp.(po/r)); copies split across ScalarE and DVE
        has = []
        for t in range(CT):
            ha = spool.tile([P, CHW], BF16, tag=f"hx{t}", bufs=2)
            if t < 2:
                nc.scalar.copy(ha[:], po[t][:])
            else:
                nc.vector.tensor_copy(ha[:], po[t][:])
            has.append(ha)
        nc.vector.tensor_copy(rb[:], prb[:])
        for ot in range(CT):
            # reuse the freed po slot: the pa slots stay available for the
            # next half's score pipeline even while the 1/r chain lags
            py = qpool.tile([P, CHW], F32, tag=f"po{ot}", name="py", bufs=1)
            for t in range(CT):
                nc.tensor.matmul(py[:], wts["wpT"][t][:, ot * P:(ot + 1) * P],
                                 has[t][:], start=(t == 0), stop=(t == CT - 1))
            # column-split epilogue: each half fires its y DMA as soon as
            # its two DVE ops are done, hiding the per-transfer DMA latency
            yt = spool.tile([P, CHW], F32, tag="yt", bufs=3)
            for hc in range(2):
                cs = slice(hc * (CHW // 2), (hc + 1) * (CHW // 2))
                t1 = spool.tile([P, CHW // 2], F32, tag="t1", bufs=2)
                nc.vector.tensor_tensor(t1[:], py[:, cs], rb[:, cs],
                                        op=OP.mult)
                nc.vector.tensor_tensor(yt[:, cs], t1[:], xyb[ih][ot][:, cs],
                                        op=OP.add)
                (nc.gpsimd if (2 * ot + hc) % 2 == 0 else nc.sync).dma_start(
                    y_d[ot, :, ih * CHW + hc * (CHW // 2):
                        ih * CHW + (hc + 1) * (CHW // 2)], yt[:, cs])

    # scores/exp run LA j-groups ahead of the AV and LR groups ahead of the
    # rowsum that consume the exp output: by the time the PE reaches each
    # consumer, the exp's (late-posting) completion semaphore is stale and
    # the PE never waits on ScalarE. The next half's first KPRE score
    # groups are emitted into the drain/tail window so the PE never idles
    # across halves.
    LA, KPRE = 3, 5
    pr0 = mk_pr()
    po0 = mk_po()
    pr1 = mk_pr()
    q0, pre = {}, {}
    for it in range(JT + KPRE):
        if it < JT:
            q0[it] = sc_exp(0, it)
        elif it - JT < KPRE:
            pre[it - JT] = sc_exp(1, it - JT)
        # rowsums in bursts of 8: the M=1 matmul costs ~90ns of PE
        # reconfig on each entry/exit, so amortize it; the last burst
        # lands before the last AV so the 1/r chain overlaps it
        if it > 0 and it % 16 == 0:
            for jj in range(it - 16, it):
                rowsum_only(pr0, jj, q0[jj])
        ja = it - LA
        if 0 <= ja < JT:
            av_only(po0, ja, q0[ja])
    tail_and_y(pr0, po0, 0)
    po1 = mk_po()
    for it in range(JT):
        js = it + KPRE
        if js < JT:
            pre[js] = sc_exp(1, js)
        if it % 16 == 15:
            for jj in range(it - 15, it + 1):
                rowsum_only(pr1, jj, pre[jj])
        av_only(po1, it, pre[it])
    tail_and_y(pr1, po1, 1)


def build_module():
    nc = bacc.Bacc("TRN2", target_bir_lowering=False, debug=False,
                   num_devices=NCORES)
    d = {
        "xb": nc.dram_tensor("xb", [CT, 4, P, QN], BF16,
                             kind="ExternalInput").ap(),
        "wqT": nc.dram_tensor("wqT", [CT, P, C], BF16, kind="ExternalInput").ap(),
        "wkT": nc.dram_tensor("wkT", [CT, P, C], BF16, kind="ExternalInput").ap(),
        "wvT": nc.dram_tensor("wvT", [CT, P, C], BF16, kind="ExternalInput").ap(),
        "wpT": nc.dram_tensor("wpT", [CT, P, C], BF16, kind="ExternalInput").ap(),
        "chv": nc.dram_tensor("chv", [P, 6 * CT], F32, kind="ExternalInput").ap(),
        "ind": nc.dram_tensor("ind", [P, GPT], F32, kind="ExternalInput").ap(),
        "indb": nc.dram_tensor("indb", [P, GPT], BF16,
                               kind="ExternalInput").ap(),
        "indT": nc.dram_tensor("indT", [GPT, P], F32, kind="ExternalInput").ap(),
        "y": nc.dram_tensor("y", [CT, P, QN], F32, kind="ExternalOutput").ap(),
    }
    with tile.TileContext(nc) as tc, ExitStack() as ctx:
        _build_body(nc, tc, ctx, d)
    nc.compile()
    return nc


_CACHE = {}


def _get_nc():
    if "nc" not in _CACHE:
        _CACHE["nc"] = build_module()
    return _CACHE["nc"]


def _shared_inputs(gamma, beta, wq, bq, wk, bk, wv, bv, wp, bp):
    import ml_dtypes

    def wT(w):
        wt = np.ascontiguousarray(np.asarray(w, np.float32).T)
        return wt.reshape(CT, P, C).astype(ml_dtypes.bfloat16)

    ind = np.zeros((P, GPT), np.float32)
    for i in range(P):
        ind[i, i // 16] = 1.0 / 16.0
    indT = np.zeros((GPT, P), np.float32)
    for i in range(P):
        indT[i // 16, i] = 1.0
    chv = np.stack([np.asarray(a, np.float32)
                    for a in (gamma, beta, bq, bk, bv, bp)],
                   axis=1).reshape(CT, P, 6).transpose(1, 0, 2).reshape(P, 24)
    return {
        "wqT": wT(wq), "wkT": wT(wk), "wvT": wT(wv), "wpT": wT(wp),
        "chv": np.ascontiguousarray(chv),
        "ind": ind, "indb": ind.astype(ml_dtypes.bfloat16), "indT": indT,
    }


def make_in_maps(x, gamma, beta, wq, bq, wk, bk, wv, bv, wp, bp):
    import ml_dtypes

    shared = _shared_inputs(gamma, beta, wq, bq, wk, bk, wv, bv, wp, bp)
    xf = np.asarray(x, np.float32).reshape(B, C, HW)
    in_maps = []
    for core in range(NCORES):
        b, qc = divmod(core, NCORES // B)
        xb = np.roll(xf[b], -qc * QN, axis=1)          # [C, HW]
        xt = xb.reshape(CT, P, 4, QN).transpose(0, 2, 1, 3)
        m = dict(shared)
        m["xb"] = np.ascontiguousarray(xt).astype(ml_dtypes.bfloat16)
        in_maps.append(m)
    return in_maps


def assemble_output(results):
    out = np.empty((B, C, HW), np.float32)
    for core in range(NCORES):
        b, qc = divmod(core, NCORES // B)
        y = np.asarray(results[core]["y"]).reshape(C, QN)
        out[b, :, qc * QN:(qc + 1) * QN] = y
    return out.reshape(B, C, H, W)


def kernel(x, gamma, beta, wq, bq, wk, bk, wv, bv, wp, bp):
    nc = _get_nc()
    in_maps = make_in_maps(x, gamma, beta, wq, bq, wk, bk, wv, bv, wp, bp)
    res = run_bass_kernel_spmd(nc, in_maps, list(range(NCORES)))
    return assemble_output(res.results)



# revision 6
# speedup vs baseline: 1.3980x; 1.3980x over previous
"""Trainium2 Bass kernel for nn_AttentionBlock (GroupNorm + single-head spatial
self-attention + residual) on 8 NeuronCores.

Sharding: data-parallel over batch (2) x sequence-parallel over the query
dimension (4 chunks of 1024 of the 4096 spatial tokens). Each core gets the
full image of its batch element, ROTATED so its query chunk sits at token 0
(GroupNorm stats, key/value sets and softmax sums are permutation-invariant
over tokens, so rotation lets all 8 cores run the identical SPMD program).

v2: every large matmul runs in fp8e4m3 with perf_mode=DoubleRow (two K=128
contraction tiles fused per instruction, ~1.44x PE throughput at FD>=512).
All pairs are [128, 2, N] tiles pairing adjacent 128-blocks of the
contraction dim. Measured end-to-end rel err ~9e-3 vs the 2e-2 gate.

  x ships as fp8 channel-pairs (2MB) and is the matmul operand for q/k/v;
  a second bf16 copy of x arrives later (DMA idle mid-kernel) only for the
  residual. GroupNorm stats come from the fp8 x via DVE free-dim reduces
  (sums) + ScalarE Square activations with accum_out (sumsq) streamed
  behind the DMAs; group-combine via tiny PE indicator matmuls as before.
  The GroupNorm scale a[c] folds into fp8 copies of wq/wk/wv (bf16 raw
  weights ship for the tiny bias contracts); the shift b[c] becomes
  per-output-channel constants (rows of w.b) as in v1.

  scores: sT[j,i] = sum_c k[c,j] q[c,i] as 2 DoubleRow matmuls (c-pairs);
  exp on ScalarE with a constant logit shift of -2.5 (softmax is shift
  invariant; the shift keeps exp under fp8 max 240; logit max is ~7.2)
  writing fp8 straight into j-pair tiles. AV + rowsum consume the pairs
  with DoubleRow (vT j-pairs / fp8 ones). Softmax normalization is
  deferred past the wp projection: po -> fp8 'has' pairs scaled by 1/16
  (keeps po under fp8 range; the x16 folds into the 1/r row broadcast,
  whose ones-row carries value 16). Final y = py*(16/r) + (x + yb).
"""

import sys
from contextlib import ExitStack

if "/opt/trn_rl_repo" not in sys.path:
    sys.path.insert(0, "/opt/trn_rl_repo")

import numpy as np

import concourse.bass as bass  # noqa: F401  (import keeps bass registered)
import concourse.tile as tile
from concourse import bacc, mybir
from concourse.alu_op_type import AluOpType
from concourse.bass_utils import run_bass_kernel_spmd

F32 = mybir.dt.float32
BF16 = mybir.dt.bfloat16
FP8 = mybir.dt.float8e4
AF = mybir.ActivationFunctionType
OP = AluOpType
DR = mybir.MatmulPerfMode.DoubleRow
AXX = mybir.AxisListType.X

B, C, H, W = 2, 512, 64, 64
HW = H * W          # 4096 spatial tokens
P = 128             # partitions
CT = C // P         # 4 channel tiles
PT = CT // 2        # 2 channel pair-tiles
NCORES = 8
QN = HW // 4        # 1024 queries per core
CHW = 512           # token chunk width
JT = HW // P        # 32 key tiles
JJ = JT // 2        # 16 key pair-tiles
EPS = 1e-6
SCALE = float(C) ** -0.5
SH = 2.5            # constant logit shift (softmax-invariant; fp8 range)
POS = 1.0 / 16.0    # po -> fp8 scale; 16x folds into the 1/r broadcast
GPT = P // 16       # 8 groups per channel tile


def _build_body(nc, tc, ctx, d):
    x8_d = d["x8"]
    xb_d = d["xb"]
    y_d = d["y"]

    cpool = ctx.enter_context(tc.tile_pool(name="const", bufs=1))
    ppool = ctx.enter_context(tc.tile_pool(name="persist", bufs=1))
    spool = ctx.enter_context(tc.tile_pool(name="stream", bufs=2))
    smpool = ctx.enter_context(tc.tile_pool(name="small", bufs=1))
    qpool = ctx.enter_context(tc.tile_pool(name="psum", bufs=3, space="PSUM"))

    dma_engines = [nc.gpsimd, nc.scalar, nc.sync]

    # ---- phase 1: x arrives fp8 in channel-pair layout (quarter transfers,
    # earliest tokens first) and is the matmul operand for the whole kernel
    x8 = [ppool.tile([P, 2, HW], FP8, tag=f"x8{t}", name=f"x8{t}")
          for t in range(PT)]
    for qtr in range(4):
        for t in range(PT):
            eng = dma_engines[(qtr * PT + t) % 3]
            eng.dma_start(x8[t][:, :, qtr * QN:(qtr + 1) * QN], x8_d[t, qtr])
    # small constants right behind the x stream
    ind = cpool.tile([P, GPT], F32, tag="ind")
    nc.gpsimd.dma_start(ind[:], d["ind"][:])
    indT = cpool.tile([GPT, P], F32, tag="indT")
    nc.sync.dma_start(indT[:], d["indT"][:])
    chv24 = cpool.tile([P, 6 * CT], F32, tag="chv24")
    nc.gpsimd.dma_start(chv24[:], d["chv"][:])
    # chv columns per tile t at 6*t+j: gamma, beta, bq, bk, bv, bp
    chv3 = chv24.rearrange("p (t six) -> p t six", six=6)

    def chvcol(t, c):
        return chv24[:, 6 * t + c:6 * t + c + 1]
    # bulk weights, in consumption order, spread across the 3 DMA queues
    wts = {}
    for wi, name in enumerate(("wkT", "wvT", "wqT")):
        wts[name] = []
        for t in range(CT):
            w = cpool.tile([P, C], BF16, tag=f"{name}{t}", name=f"{name}{t}")
            dma_engines[(wi * CT + t) % 3].dma_start(w[:], d[name][t])
            wts[name].append(w)
    wp8 = []
    for t in range(PT):
        w = cpool.tile([P, 2, C], FP8, tag=f"wp8{t}", name=f"wp8{t}")
        dma_engines[t % 3].dma_start(w[:], d["wp8"][t])
        wp8.append(w)
    # bf16 x for the residual path only -- needed from the xyb precompute
    # on, so it rides last on the DMA queues
    xb_sb = [ppool.tile([P, HW], BF16, tag=f"xb{t}", name=f"xb{t}")
             for t in range(CT)]
    for t in range(CT):
        for qtr in range(4):
            dma_engines[(t * 4 + qtr) % 3].dma_start(
                xb_sb[t][:, qtr * QN:(qtr + 1) * QN], xb_d[t, qtr])

    ones8 = cpool.tile([P, 2, 16], FP8, tag="ones8")
    nc.vector.memset(ones8[:], 1.0)
    ones_row = smpool.tile([1, P], BF16, tag="onesr")
    nc.vector.memset(ones_row[:], 16.0)   # carries the x16 of the po scale
    epst = smpool.tile([GPT, CT], F32, tag="eps")
    nc.vector.memset(epst[:], EPS)
    ebias = smpool.tile([P, 1], F32, tag="ebias")
    nc.vector.memset(ebias[:], -SH)
    pos16 = smpool.tile([P, 1], F32, tag="pos16")
    nc.vector.memset(pos16[:], POS)
    dumt = smpool.tile([GPT, 1], F32, tag="dumt")

    # GroupNorm stats: per quarter-transfer (t, qtr) covering both parities,
    # DVE free-dim reduces for sums and ScalarE Square+accum for sumsq,
    # streaming behind the DMAs. sums/sumsq land in per-(old)tile columns.
    sums = smpool.tile([P, CT, 4], F32, tag="sums")
    sumsq = smpool.tile([P, CT, 4], F32, tag="sumsq")
    garb = smpool.tile([P, QN], FP8, tag="garb", bufs=2)
    for qtr in range(4):
        for t in range(PT):
            for e in range(2):
                ot = 2 * t + e
                sl = x8[t][:, e, qtr * QN:(qtr + 1) * QN]
                nc.vector.tensor_reduce(sums[:, ot, qtr:qtr + 1], sl, AXX,
                                        OP.add)
                nc.scalar.activation(garb[:], sl, AF.Square,
                                     accum_out=sumsq[:, ot, qtr:qtr + 1])
    # preload the Sqrt table while the DVE combine chain runs (the Square
    # activations above are done with the table at this point)
    nc.scalar.activation(dumt[:], epst[:, 0:1], AF.Sqrt)
    # s4 holds [sum, sumsq] per tile; the 1/(16*HW) group-mean scale is
    # folded into the ind indicator values (host side)
    s4 = smpool.tile([P, CT, 2], F32, tag="s4")
    nc.vector.tensor_reduce(s4[:, :, 0], sums[:], AXX, OP.add)
    nc.vector.tensor_reduce(s4[:, :, 1], sumsq[:], AXX, OP.add)
    gps = qpool.tile([GPT, 2 * CT], F32, tag="pa")
    for t in range(CT):
        nc.tensor.matmul(gps[:, 2 * t:2 * t + 2], ind[:], s4[:, t, :],
                         start=True, stop=True)
    gst = smpool.tile([GPT, 2 * CT], F32, tag="gst")
    nc.vector.tensor_copy(gst[:], gps[:])
    g3 = gst.rearrange("p (t two) -> p t two", two=2)
    mu = smpool.tile([GPT, CT], F32, tag="mu")
    nc.vector.tensor_copy(mu[:], g3[:, :, 0])
    msq = smpool.tile([GPT, CT], F32, tag="msq")
    nc.vector.tensor_tensor(msq[:], mu[:], mu[:], op=OP.mult)
    varg = smpool.tile([GPT, CT], F32, tag="varg")
    nc.vector.tensor_tensor(varg[:], g3[:, :, 1], msq[:], op=OP.subtract)
    stdg = smpool.tile([GPT, CT], F32, tag="stdg")
    nc.scalar.activation(stdg[:], varg[:], AF.Sqrt, bias=epst[:, 0:1])
    # preload the Exp table right after the last Sqrt (copies in between
    # are table-neutral), so phase 3's first exp starts without a reload
    nc.scalar.activation(dumt[:], stdg[:, 0:1], AF.Exp)
    # interleave (mu_t, rstd_t) columns and broadcast all groups->channels
    # with a single [K=8, M=128, N=8] indicator matmul
    mr = smpool.tile([GPT, 2 * CT], F32, tag="mr")
    mr3 = mr.rearrange("p (t two) -> p t two", two=2)
    nc.vector.tensor_copy(mr3[:, :, 0], mu[:])
    nc.vector.reciprocal(mr3[:, :, 1], stdg[:])
    cba = qpool.tile([P, 2 * CT], F32, tag="pa")
    nc.tensor.matmul(cba[:], indT[:], mr[:], start=True, stop=True)
    cb = smpool.tile([P, 2 * CT], F32, tag="cb")
    nc.vector.tensor_copy(cb[:], cba[:])

    # per-channel Scale a / Bias b, vectorized across all 4 tiles via
    # strided views (one wide DVE op instead of one per tile)
    cb2 = cb.rearrange("p (t two) -> p t two", two=2)
    ab = ppool.tile([P, 2, CT], F32, tag="ab")   # [:,0,t]=a_t  [:,1,t]=b_t
    nc.vector.tensor_tensor(ab[:, 0, :], cb2[:, :, 1], chv3[:, :, 0],
                            op=OP.mult)
    tmpb = smpool.tile([P, CT], F32, tag="tmpb", bufs=1)
    nc.vector.tensor_tensor(tmpb[:], cb2[:, :, 0], ab[:, 0, :], op=OP.mult)
    nc.vector.tensor_tensor(ab[:, 1, :], chv3[:, :, 1], tmpb[:],
                            op=OP.subtract)
    bvec_all = ppool.tile([P, CT, 2], BF16, tag="bva")
    nc.vector.tensor_copy(bvec_all[:, :, 0], ab[:, 1, :])
    nc.vector.tensor_copy(bvec_all[:, :, 1], ab[:, 1, :])
    sbts = [ab[:, 0, t:t + 1] for t in range(CT)]
    bvec = [bvec_all[:, t, :] for t in range(CT)]

    # GroupNorm scale folded into fp8 PAIR COPIES of the projection weights
    # (the raw bf16 tiles stay live for the bias contracts interleaved into
    # phase 2 below)
    ws8 = {}
    for name in ("wkT", "wvT", "wqT"):
        ws8[name] = []
        for t in range(PT):
            w = cpool.tile([P, 2, C], FP8, tag=f"s{name}{t}",
                           name=f"s{name}{t}")
            for e in range(2):
                nc.vector.tensor_scalar_mul(w[:, e, :], wts[name][2 * t + e][:],
                                            sbts[2 * t + e])
            ws8[name].append(w)

    # ---- bias-term constants from RAW weights (tiny N=2 matmuls);
    # emitted one output-tile group at a time, interleaved into phase 2's
    # first chunks so they never head-of-line block the projections ----
    #   qb[o] = sum_c wq[o,c] b[c] + bq    (added at the q PSUM->SBUF move)
    #   kb[o] = likewise with bk
    #   vb8[c] = sum_cin wv[c,cin] b[cin] + bv   (rides softmax into yb)
    #   yb[o] = sum_c wp[o,c] vb8[c] + bp        (y epilogue constant)
    vb8 = [ppool.tile([P, 2, 16], FP8, tag=f"vb8{t}", name=f"vb8{t}")
           for t in range(PT)]

    def bias_ct(wname, ot, outdt, addcol, tagp):
        pb = qpool.tile([P, 2], F32, tag="pa", name="pb")
        for t in range(CT):
            nc.tensor.matmul(pb[:], wts[wname][t][:, ot * P:(ot + 1) * P],
                             bvec[t][:, 0:2], start=(t == 0),
                             stop=(t == CT - 1))
        if outdt == F32:
            ob = ppool.tile([P, 1], F32, tag=f"{tagp}{ot}", name=f"{tagp}{ot}")
            nc.vector.tensor_scalar(ob[:], pb[:, 0:1], chvcol(ot, addcol),
                                    None, OP.add)
            return ob
        # fp8 pair column for the yb contract
        tf = smpool.tile([P, 1], F32, tag="tf", bufs=2)
        nc.vector.tensor_scalar(tf[:], pb[:, 0:1], chvcol(ot, addcol),
                                None, OP.add)
        nc.vector.tensor_copy(vb8[ot // 2][:, ot % 2, 0:1], tf[:])
        nc.vector.tensor_copy(vb8[ot // 2][:, ot % 2, 1:2], tf[:])
        return None

    def yb_ct(ot):
        pb = qpool.tile([P, 2], F32, tag="pa", name="pb")
        for t in range(PT):
            nc.tensor.matmul(pb[:], wp8[t][:, :, ot * P:(ot + 1) * P],
                             vb8[t][:, :, 0:2], start=(t == 0),
                             stop=(t == PT - 1), perf_mode=DR)
        ob = ppool.tile([P, 1], F32, tag=f"yb{ot}", name=f"yb{ot}")
        nc.vector.tensor_scalar(ob[:], pb[:, 0:1], chvcol(ot, 5),
                                None, OP.add)
        return ob

    # ---- persistent attention operands (all fp8 pairs) ----
    k8 = [ppool.tile([P, 2, HW], FP8, tag=f"k8{t}", name=f"k8{t}")
          for t in range(PT)]
    q8 = [ppool.tile([P, 2, QN], FP8, tag=f"q8{t}", name=f"q8{t}")
          for t in range(PT)]
    vT8 = [ppool.tile([P, 2, C], FP8, tag=f"vT8{j}", name=f"vT8{j}")
           for j in range(JJ)]

    # ---- phase 2: q/k/v projections straight from resident fp8 x; the
    # bias contracts ride along inside chunks 0-1 ----
    kb, qb, yb = [None] * CT, [None] * CT, [None] * CT
    p2tags = ["pa", "po0", "po1", "po2", "po3"]
    p2cnt = [0]

    def p2psum():
        tag = p2tags[p2cnt[0] % len(p2tags)]
        p2cnt[0] += 1
        return qpool.tile([P, CHW], F32, tag=tag,
                          bufs=(3 if tag == "pa" else 1), name="p2")

    for ch in range(HW // CHW):
        sl = slice(ch * CHW, (ch + 1) * CHW)
        for ot in range(CT):
            pk = p2psum()
            for t in range(PT):
                nc.tensor.matmul(pk[:], ws8["wkT"][t][:, :, ot * P:(ot + 1) * P],
                                 x8[t][:, :, sl], start=(t == 0),
                                 stop=(t == PT - 1), perf_mode=DR)
            if ch == 0:
                kb[ot] = bias_ct("wkT", ot, F32, 3, "kb")
            nc.vector.tensor_scalar(k8[ot // 2][:, ot % 2, sl], pk[:],
                                    kb[ot][:, 0:1], None, OP.add)
        for nt in range(CT):
            jt = ch * CT + nt
            pv = p2psum()
            for t in range(PT):
                nc.tensor.matmul(pv[:], x8[t][:, :, ch * CHW + nt * P:
                                              ch * CHW + (nt + 1) * P],
                                 ws8["wvT"][t][:], start=(t == 0),
                                 stop=(t == PT - 1), perf_mode=DR)
            if ch == 0:
                bias_ct("wvT", nt, FP8, 4, "vbt")
            elif ch == 1:
                yb[nt] = yb_ct(nt)
            nc.scalar.copy(vT8[jt // 2][:, jt % 2, :], pv[:])
        if ch * CHW < QN:
            for ot in range(CT):
                pq = p2psum()
                for t in range(PT):
                    nc.tensor.matmul(pq[:],
                                     ws8["wqT"][t][:, :, ot * P:(ot + 1) * P],
                                     x8[t][:, :, sl], start=(t == 0),
                                     stop=(t == PT - 1), perf_mode=DR)
                if ch == 0:
                    qb[ot] = bias_ct("wqT", ot, F32, 2, "qb")
                nc.vector.tensor_scalar(q8[ot // 2][:, ot % 2, sl], pq[:],
                                        qb[ot][:, 0:1], None, OP.add)

    # x + yb, precomputed off the critical path for the y epilogue
    xyb = [[None] * CT for _ in range(2)]
    for ih in range(2):
        for ot in range(CT):
            xt = ppool.tile([P, CHW], F32, tag=f"xyb{ih}{ot}",
                            name=f"xyb{ih}{ot}")
            nc.vector.tensor_scalar(xt[:],
                                    xb_sb[ot][:, ih * CHW:(ih + 1) * CHW],
                                    yb[ot][:, 0:1], None, OP.add)
            xyb[ih][ot] = xt

    # ---- phase 3: attention, per query half ----
    def mk_pr():
        return qpool.tile([1, CHW], F32, tag="pr", bufs=1, name="pr")

    def mk_po():
        return [qpool.tile([P, CHW], F32, tag=f"po{t}", name=f"po{t}", bufs=1)
                for t in range(CT)]

    def sc_exp(ih, j):
        isl = slice(ih * CHW, (ih + 1) * CHW)
        ps_ = qpool.tile([P, CHW], F32, tag="pa", name="ps")
        for t in range(PT):
            nc.tensor.matmul(ps_[:], k8[t][:, :, j * P:(j + 1) * P],
                             q8[t][:, :, isl], start=(t == 0),
                             stop=(t == PT - 1), perf_mode=DR)
        pT = spool.tile([P, 2, CHW], FP8, tag="pT", bufs=12, name="pT") \
            if j % 2 == 0 else None
        return ps_, pT

    def exp_into(pair, par, ps_):
        nc.scalar.activation(pair[:, par, :], ps_[:], AF.Exp, scale=SCALE,
                             bias=ebias[:, 0:1])

    def av_only(po, jj, pair):
        for t in range(CT):
            nc.tensor.matmul(po[t][:], vT8[jj][:, :, t * P:(t + 1) * P],
                             pair[:], start=(jj == 0), stop=(jj == JJ - 1),
                             perf_mode=DR)

    def rowsum_only(pr, jj, pair):
        nc.tensor.matmul(pr[:], ones8[:, :, 0:1], pair[:],
                         start=(jj == 0), stop=(jj == JJ - 1), perf_mode=DR)

    def tail_and_y(pr, po, ih):
        # 1/r chain first: pr closed early (rowsum bursts), so DVE computes
        # rinv/rbb BEFORE the last AV finishes and the broadcast matmul
        # issues at tail start instead of after the first wp chain
        rinv = smpool.tile([1, CHW], F32, tag="rinv", bufs=2)
        nc.vector.reciprocal_approx_fast(rinv[:], pr[:])
        rbb = smpool.tile([1, CHW], BF16, tag="rbb", bufs=2)
        nc.vector.tensor_copy(rbb[:], rinv[:])
        rb = spool.tile([P, CHW], F32, tag="rb", bufs=2)
        prb = qpool.tile([P, CHW], F32, tag="pa")
        nc.tensor.matmul(prb[:], ones_row[:], rbb[:], start=True, stop=True)
        # unnormalized h -> fp8/16 so the wp matmuls start immediately
        # ((wp.po)/r == wp.(po/r)); all on DVE: GpSimd can't read PSUM and
        # activation-Copy on ScalarE would thrash the Exp table
        has = []
        for t in range(PT):
            ha = spool.tile([P, 2, CHW], FP8, tag=f"hx{t}", bufs=2)
            nc.vector.tensor_scalar(ha[:, 0, :], po[2 * t][:],
                                    pos16[:, 0:1], None, OP.mult)
            nc.vector.tensor_scalar(ha[:, 1, :], po[2 * t + 1][:],
                                    pos16[:, 0:1], None, OP.mult)
            has.append(ha)
        nc.vector.tensor_copy(rb[:], prb[:])
        for ot in range(CT):
            # reuse the freed po slot: the pa slots stay available for the
            # next half's score pipeline even while the 1/r chain lags
            py = qpool.tile([P, CHW], F32, tag=f"po{ot}", name="py", bufs=1)
            for t in range(PT):
                nc.tensor.matmul(py[:], wp8[t][:, :, ot * P:(ot + 1) * P],
                                 has[t][:], start=(t == 0),
                                 stop=(t == PT - 1), perf_mode=DR)
            # column-split epilogue: each half fires its y DMA as soon as
            # its two DVE ops are done, hiding the per-transfer DMA latency
            yt = spool.tile([P, CHW], F32, tag="yt", bufs=3)
            for hc in range(2):
                cs = slice(hc * (CHW // 2), (hc + 1) * (CHW // 2))
                t1 = spool.tile([P, CHW // 2], F32, tag="t1", bufs=2)
                nc.vector.tensor_tensor(t1[:], py[:, cs], rb[:, cs],
                                        op=OP.mult)
                nc.vector.tensor_tensor(yt[:, cs], t1[:], xyb[ih][ot][:, cs],
                                        op=OP.add)
                (nc.gpsimd if (2 * ot + hc) % 2 == 0 else nc.sync).dma_start(
                    y_d[ot, :, ih * CHW + hc * (CHW // 2):
                        ih * CHW + (hc + 1) * (CHW // 2)], yt[:, cs])

    # scores/exp run LA jj-pairs ahead of the AV that consumes the pair:
    # by the time the PE reaches each consumer, the exp's (late-posting)
    # completion semaphore is stale and the PE never waits on ScalarE. The
    # next half's first KPRE pair groups are emitted into the drain/tail
    # window so the PE never idles across halves.
    LA, KPRE = 2, 3
    pr0 = mk_pr()
    po0 = mk_po()
    pr1 = mk_pr()
    pairs0, pre = {}, {}
    for it in range(JJ + KPRE):
        if it < JJ:
            ps0, pair = sc_exp(0, 2 * it)
            exp_into(pair, 0, ps0)
            ps1, _ = sc_exp(0, 2 * it + 1)
            exp_into(pair, 1, ps1)
            pairs0[it] = pair
        elif it - JJ < KPRE:
            jjp = it - JJ
            ps0, pair = sc_exp(1, 2 * jjp)
            exp_into(pair, 0, ps0)
            ps1, _ = sc_exp(1, 2 * jjp + 1)
            exp_into(pair, 1, ps1)
            pre[jjp] = pair
        # rowsums in bursts of 8 pairs: the M=1 matmul costs PE reconfig on
        # each entry/exit, so amortize it; the last burst lands before the
        # last AV so the 1/r chain overlaps it
        if it in (JJ // 2, JJ):
            for jp in range(it - JJ // 2, it):
                rowsum_only(pr0, jp, pairs0[jp])
        ja = it - LA
        if 0 <= ja < JJ:
            av_only(po0, ja, pairs0[ja])
    tail_and_y(pr0, po0, 0)
    po1 = mk_po()
    for it in range(JJ):
        js = it + KPRE
        if js < JJ:
            ps0, pair = sc_exp(1, 2 * js)
            exp_into(pair, 0, ps0)
            ps1, _ = sc_exp(1, 2 * js + 1)
            exp_into(pair, 1, ps1)
            pre[js] = pair
        if it in (JJ // 2 - 1, JJ - 1):
            for jp in range(it - JJ // 2 + 1, it + 1):
                rowsum_only(pr1, jp, pre[jp])
        av_only(po1, it, pre[it])
    tail_and_y(pr1, po1, 1)


def build_module():
    nc = bacc.Bacc("TRN2", target_bir_lowering=False, debug=False,
                   num_devices=NCORES)
    d = {
        "x8": nc.dram_tensor("x8", [PT, 4, P, 2, QN], FP8,
                             kind="ExternalInput").ap(),
        "xb": nc.dram_tensor("xb", [CT, 4, P, QN], BF16,
                             kind="ExternalInput").ap(),
        "wqT": nc.dram_tensor("wqT", [CT, P, C], BF16, kind="ExternalInput").ap(),
        "wkT": nc.dram_tensor("wkT", [CT, P, C], BF16, kind="ExternalInput").ap(),
        "wvT": nc.dram_tensor("wvT", [CT, P, C], BF16, kind="ExternalInput").ap(),
        "wp8": nc.dram_tensor("wp8", [PT, P, 2, C], FP8,
                              kind="ExternalInput").ap(),
        "chv": nc.dram_tensor("chv", [P, 6 * CT], F32, kind="ExternalInput").ap(),
        "ind": nc.dram_tensor("ind", [P, GPT], F32, kind="ExternalInput").ap(),
        "indT": nc.dram_tensor("indT", [GPT, P], F32, kind="ExternalInput").ap(),
        "y": nc.dram_tensor("y", [CT, P, QN], F32, kind="ExternalOutput").ap(),
    }
    with tile.TileContext(nc) as tc, ExitStack() as ctx:
        _build_body(nc, tc, ctx, d)
    nc.compile()
    return nc


_CACHE = {}


def _get_nc():
    if "nc" not in _CACHE:
        _CACHE["nc"] = build_module()
    return _CACHE["nc"]


def _shared_inputs(gamma, beta, wq, bq, wk, bk, wv, bv, wp, bp):
    import ml_dtypes

    def wT(w):
        wt = np.ascontiguousarray(np.asarray(w, np.float32).T)
        return wt.reshape(CT, P, C).astype(ml_dtypes.bfloat16)

    wpT = np.asarray(wp, np.float32).T.reshape(PT, 2, P, C)
    wp8 = np.ascontiguousarray(wpT.transpose(0, 2, 1, 3)).astype(
        ml_dtypes.float8_e4m3)

    ind = np.zeros((P, GPT), np.float32)
    for i in range(P):
        ind[i, i // 16] = 1.0 / (16.0 * HW)
    indT = np.zeros((GPT, P), np.float32)
    for i in range(P):
        indT[i // 16, i] = 1.0
    chv = np.stack([np.asarray(a, np.float32)
                    for a in (gamma, beta, bq, bk, bv, bp)],
                   axis=1).reshape(CT, P, 6).transpose(1, 0, 2).reshape(P, 24)
    return {
        "wqT": wT(wq), "wkT": wT(wk), "wvT": wT(wv), "wp8": wp8,
        "chv": np.ascontiguousarray(chv),
        "ind": ind, "indT": indT,
    }


def make_in_maps(x, gamma, beta, wq, bq, wk, bk, wv, bv, wp, bp):
    import ml_dtypes

    shared = _shared_inputs(gamma, beta, wq, bq, wk, bk, wv, bv, wp, bp)
    xf = np.asarray(x, np.float32).reshape(B, C, HW)
    in_maps = []
    for core in range(NCORES):
        b, qc = divmod(core, NCORES // B)
        xr = np.roll(xf[b], -qc * QN, axis=1)          # [C, HW]
        # fp8 channel-pair layout [T, qtr, p, e, m]: c = 256T+128e+p
        x8 = xr.reshape(PT, 2, P, 4, QN).transpose(0, 3, 2, 1, 4)
        xt = xr.reshape(CT, P, 4, QN).transpose(0, 2, 1, 3)
        m = dict(shared)
        m["x8"] = np.ascontiguousarray(x8).astype(ml_dtypes.float8_e4m3)
        m["xb"] = np.ascontiguousarray(xt).astype(ml_dtypes.bfloat16)
        in_maps.append(m)
    return in_maps


def assemble_output(results):
    out = np.empty((B, C, HW), np.float32)
    for core in range(NCORES):
        b, qc = divmod(core, NCORES // B)
        y = np.asarray(results[core]["y"]).reshape(C, QN)
        out[b, :, qc * QN:(qc + 1) * QN] = y
    return out.reshape(B, C, H, W)


def kernel(x, gamma, beta, wq, bq, wk, bk, wv, bv, wp, bp):
    nc = _get_nc()
    in_maps = make_in_maps(x, gamma, beta, wq, bq, wk, bk, wv, bv, wp, bp)
    res = run_bass_kernel_spmd(nc, in_maps, list(range(NCORES)))
    return assemble_output(res.results)


# revision 11
# speedup vs baseline: 1.6503x; 1.1805x over previous
"""Trainium2 Bass kernel for nn_AttentionBlock (GroupNorm + single-head spatial
self-attention + residual) on 8 NeuronCores.

Sharding: data-parallel over batch (2) x sequence-parallel over the query
dimension (4 chunks of 1024 of the 4096 spatial tokens). Each core gets the
full image of its batch element, ROTATED so its query chunk sits at token 0
(GroupNorm stats, key/value sets and softmax sums are permutation-invariant
over tokens, so rotation lets all 8 cores run the identical SPMD program).

v2: every large matmul runs in fp8e4m3 with perf_mode=DoubleRow (two K=128
contraction tiles fused per instruction, ~1.44x PE throughput at FD>=512).
All pairs are [128, 2, N] tiles pairing adjacent 128-blocks of the
contraction dim. Measured end-to-end rel err ~9e-3 vs the 2e-2 gate.

  x ships as fp8 channel-pairs (2MB) and is the matmul operand for q/k/v;
  a second bf16 copy of x arrives later (DMA idle mid-kernel) only for the
  residual. GroupNorm stats come from the fp8 x via DVE free-dim reduces
  (sums) + ScalarE Square activations with accum_out (sumsq) streamed
  behind the DMAs; group-combine via tiny PE indicator matmuls as before.
  The GroupNorm scale a[c] folds into fp8 copies of wq/wk/wv (bf16 raw
  weights ship for the tiny bias contracts); the shift b[c] becomes
  per-output-channel constants (rows of w.b) as in v1.

  scores: sT[j,i] = sum_c k[c,j] q[c,i] as 2 DoubleRow matmuls (c-pairs);
  exp on ScalarE with a constant logit shift of -2.5 (softmax is shift
  invariant; the shift keeps exp under fp8 max 240; logit max is ~7.2)
  writing fp8 straight into j-pair tiles. AV + rowsum consume the pairs
  with DoubleRow (vT j-pairs / fp8 ones). Softmax normalization is
  deferred past the wp projection: po -> fp8 'has' pairs scaled by 1/16
  (keeps po under fp8 range; the x16 folds into the 1/r row broadcast,
  whose ones-row carries value 16). Final y = py*(16/r) + (x + yb).
"""

import sys
from contextlib import ExitStack

if "/opt/trn_rl_repo" not in sys.path:
    sys.path.insert(0, "/opt/trn_rl_repo")

import numpy as np

import concourse.bass as bass  # noqa: F401  (import keeps bass registered)
import concourse.tile as tile
from concourse import bacc, mybir
from concourse.alu_op_type import AluOpType
from concourse.bass_utils import run_bass_kernel_spmd

F32 = mybir.dt.float32
BF16 = mybir.dt.bfloat16
FP8 = mybir.dt.float8e4
AF = mybir.ActivationFunctionType
OP = AluOpType
DR = mybir.MatmulPerfMode.DoubleRow
AXX = mybir.AxisListType.X

B, C, H, W = 2, 512, 64, 64
HW = H * W          # 4096 spatial tokens
P = 128             # partitions
CT = C // P         # 4 channel tiles
PT = CT // 2        # 2 channel pair-tiles
NCORES = 8
QN = HW // 4        # 1024 queries per core
CHW = 512           # token chunk width
JT = HW // P        # 32 key tiles
JJ = JT // 2        # 16 key pair-tiles
EPS = 1e-6
SCALE = float(C) ** -0.5
SH = 2.5            # constant logit shift (softmax-invariant; fp8 range)


def _build_body(nc, tc, ctx, d):
    x8_d = d["x8"]
    xb_d = d["xb"]
    y_d = d["y"]

    cpool = ctx.enter_context(tc.tile_pool(name="const", bufs=1))
    ppool = ctx.enter_context(tc.tile_pool(name="persist", bufs=1))
    spool = ctx.enter_context(tc.tile_pool(name="stream", bufs=2))
    smpool = ctx.enter_context(tc.tile_pool(name="small", bufs=1))
    qpool = ctx.enter_context(tc.tile_pool(name="psum", bufs=3, space="PSUM"))

    # DMA queue assignment: ScalarE's instruction queue must stay clear for
    # the phase-1 Square stream (a DMA descriptor-gen blocked on ring space
    # would stall everything behind it), so ScalarE only issues a share of
    # the x8 transfers (needed earliest); weights, consts and the low-
    # priority bf16 x ride on gpsimd/sync.
    bulk_engines = [nc.gpsimd, nc.sync]

    # ---- phase 1: x arrives fp8 in channel-pair layout (quarter transfers,
    # earliest tokens first) and is the matmul operand for the whole kernel
    x8 = [ppool.tile([P, 2, HW], FP8, tag=f"x8{t}", name=f"x8{t}")
          for t in range(PT)]
    dma3 = [nc.gpsimd, nc.scalar, nc.sync]
    for qtr in range(4):
        for t in range(PT):
            eng = dma3[(qtr * PT + t) % 3]
            eng.dma_start(x8[t][:, :, qtr * QN:(qtr + 1) * QN], x8_d[t, qtr])
    # small constants right behind the x stream
    ind8 = cpool.tile([P, 2, 16], FP8, tag="ind8")
    nc.gpsimd.dma_start(ind8[:], d["ind8"][:])
    indTA = cpool.tile([16, P], F32, tag="indTA")
    nc.sync.dma_start(indTA[:], d["indTA"][:])
    indTB = cpool.tile([16, P], F32, tag="indTB")
    nc.sync.dma_start(indTB[:], d["indTB"][:])
    chv24 = cpool.tile([P, 6 * CT], F32, tag="chv24")
    nc.gpsimd.dma_start(chv24[:], d["chv"][:])
    # chv columns per tile t at 6*t+j: gamma, beta, bq, bk, bv, bp
    chv3 = chv24.rearrange("p (t six) -> p t six", six=6)

    def chvcol(t, c):
        return chv24[:, 6 * t + c:6 * t + c + 1]
    # bulk weights, in consumption order, on the gpsimd/sync queues
    wts = {}
    for wi, name in enumerate(("wkT", "wvT", "wqT")):
        wts[name] = []
        for t in range(CT):
            w = cpool.tile([P, C], BF16, tag=f"{name}{t}", name=f"{name}{t}")
            bulk_engines[(wi * CT + t) % 2].dma_start(w[:], d[name][t])
            wts[name].append(w)
    wp8 = []
    for t in range(PT):
        w = cpool.tile([P, 2, C], FP8, tag=f"wp8{t}", name=f"wp8{t}")
        bulk_engines[t % 2].dma_start(w[:], d["wp8"][t])
        wp8.append(w)
    # bf16 x for the residual path only -- needed from the xyb precompute
    # on, so it rides last on the gpsimd/sync queues
    xb_sb = [ppool.tile([P, HW], BF16, tag=f"xb{t}", name=f"xb{t}")
             for t in range(CT)]
    for t in range(CT):
        for qtr in range(4):
            bulk_engines[(t * 4 + qtr) % 2].dma_start(
                xb_sb[t][:, qtr * QN:(qtr + 1) * QN], xb_d[t, qtr])

    ones8 = cpool.tile([P, 2, 16], FP8, tag="ones8")
    nc.vector.memset(ones8[:], 1.0)
    ones_row = smpool.tile([1, P], BF16, tag="onesr")
    nc.vector.memset(ones_row[:], 1.0)
    epst16 = smpool.tile([16, 2], F32, tag="eps")
    nc.vector.memset(epst16[:], EPS)
    ebias = smpool.tile([P, 1], F32, tag="ebias")
    nc.vector.memset(ebias[:], -SH)
    i16hw = smpool.tile([16, 1], F32, tag="i16hw")
    nc.vector.memset(i16hw[:], 1.0 / (16.0 * HW))
    dumt = smpool.tile([16, 1], F32, tag="dumt")

    # GroupNorm stats on the otherwise-idle PE: per 512-token chunk, a
    # DoubleRow matmul with a group-indicator lhsT accumulates per-group
    # token-position sums in PSUM; a second accumulator consumes x^2 chunks
    # produced by a square pass split across ScalarE (Square activation)
    # and DVE (tensor_tensor mult). Group g of pair-tile T sits at PSUM
    # row 8*parity + group-within-parity.
    psgS = [qpool.tile([16, CHW], F32, tag=f"po{T}", bufs=1, name=f"psgS{T}")
            for T in range(PT)]
    psgQ = [qpool.tile([16, CHW], F32, tag=f"po{2 + T}", bufs=1,
                       name=f"psgQ{T}") for T in range(PT)]
    for qtr in range(4):
        for T in range(PT):
            for h in range(2):
                ch = 2 * qtr + h
                sl = slice(ch * CHW, (ch + 1) * CHW)
                nc.tensor.matmul(psgS[T][:], ind8[:], x8[T][:, :, sl],
                                 start=(ch == 0), stop=(ch == 7),
                                 perf_mode=DR)
        for T in range(PT):
            for h in range(2):
                ch = 2 * qtr + h
                sl = slice(ch * CHW, (ch + 1) * CHW)
                xq = spool.tile([P, 2, CHW], FP8,
                                tag=("xsqS" if h == 0 else "xsqV"), bufs=3)
                if h == 0:
                    nc.scalar.activation(xq[:], x8[T][:, :, sl], AF.Square)
                else:
                    nc.vector.tensor_tensor(xq[:], x8[T][:, :, sl],
                                            x8[T][:, :, sl], op=OP.mult)
                nc.tensor.matmul(psgQ[T][:], ind8[:], xq[:],
                                 start=(ch == 0), stop=(ch == 7),
                                 perf_mode=DR)
    # preload the Sqrt table right behind the last Square (the combine
    # chain below hides the 1.3us load)
    nc.scalar.activation(dumt[:], epst16[:, 0:1], AF.Sqrt)
    s2 = smpool.tile([16, PT, 2], F32, tag="s2")
    for T in range(PT):
        nc.vector.tensor_reduce(s2[:, T, 0:1], psgS[T][:], AXX, OP.add)
        nc.vector.tensor_reduce(s2[:, T, 1:2], psgQ[T][:], AXX, OP.add)
    s2m = smpool.tile([16, PT, 2], F32, tag="s2m")   # [mu, E[x^2]] per T
    nc.vector.tensor_scalar(s2m[:], s2[:], i16hw[:, 0:1], None, OP.mult)
    msq = smpool.tile([16, PT], F32, tag="msq")
    nc.vector.tensor_tensor(msq[:], s2m[:, :, 0], s2m[:, :, 0], op=OP.mult)
    varg = smpool.tile([16, PT], F32, tag="varg")
    nc.vector.tensor_tensor(varg[:], s2m[:, :, 1], msq[:], op=OP.subtract)
    stdg = smpool.tile([16, PT], F32, tag="stdg")
    nc.scalar.activation(stdg[:], varg[:], AF.Sqrt, bias=epst16[:, 0:1])
    # preload the Exp table right after the last Sqrt (copies in between
    # are table-neutral), so phase 3's first exp starts without a reload
    nc.scalar.activation(dumt[:], stdg[:, 0:1], AF.Exp)
    # interleave (mu_T, rstd_T) columns and broadcast groups->channels with
    # two [K=16, M=128, N=4] indicator matmuls (one per channel parity)
    mr = smpool.tile([16, PT, 2], F32, tag="mr")
    nc.vector.tensor_copy(mr[:, :, 0], s2m[:, :, 0])
    nc.vector.reciprocal(mr[:, :, 1], stdg[:])
    cbA = qpool.tile([P, 2 * PT], F32, tag="pa")
    nc.tensor.matmul(cbA[:], indTA[:], mr[:], start=True, stop=True)
    cbB = qpool.tile([P, 2 * PT], F32, tag="pa")
    nc.tensor.matmul(cbB[:], indTB[:], mr[:], start=True, stop=True)
    # cb4[p, t, {mu,rstd}] with t = 2T + parity
    cb4 = smpool.tile([P, CT, 2], F32, tag="cb4")
    cb4v = cb4.rearrange("p (T e) two -> p T e two", e=2)
    nc.vector.tensor_copy(cb4v[:, :, 0, :], cbA[:])
    nc.vector.tensor_copy(cb4v[:, :, 1, :], cbB[:])

    # per-channel Scale a / Bias b, vectorized across all 4 tiles via
    # strided views (one wide DVE op instead of one per tile)
    ab = ppool.tile([P, 2, CT], F32, tag="ab")   # [:,0,t]=a_t  [:,1,t]=b_t
    nc.vector.tensor_tensor(ab[:, 0, :], cb4[:, :, 1], chv3[:, :, 0],
                            op=OP.mult)
    tmpb = smpool.tile([P, CT], F32, tag="tmpb", bufs=1)
    nc.vector.tensor_tensor(tmpb[:], cb4[:, :, 0], ab[:, 0, :], op=OP.mult)
    nc.vector.tensor_tensor(ab[:, 1, :], chv3[:, :, 1], tmpb[:],
                            op=OP.subtract)
    bvec_all = ppool.tile([P, CT, 2], BF16, tag="bva")
    nc.vector.tensor_copy(bvec_all[:, :, 0], ab[:, 1, :])
    nc.vector.tensor_copy(bvec_all[:, :, 1], ab[:, 1, :])
    sbts = [ab[:, 0, t:t + 1] for t in range(CT)]
    bvec = [bvec_all[:, t, :] for t in range(CT)]

    # GroupNorm scale folded into fp8 PAIR COPIES of the projection weights
    # (the raw bf16 tiles stay live for the bias contracts interleaved into
    # phase 2 below); split DVE/ScalarE (activation-Copy is table-neutral),
    # wk first so the first k-projection chunk is gated as early as possible
    ws8 = {}
    for name in ("wkT", "wvT", "wqT"):
        ws8[name] = []
        for t in range(PT):
            w = cpool.tile([P, 2, C], FP8, tag=f"s{name}{t}",
                           name=f"s{name}{t}")
            nc.vector.tensor_scalar_mul(w[:, 0, :], wts[name][2 * t][:],
                                        sbts[2 * t])
            nc.scalar.activation(w[:, 1, :], wts[name][2 * t + 1][:],
                                 AF.Copy, scale=sbts[2 * t + 1])
            ws8[name].append(w)

    # ---- bias-term constants from RAW weights (tiny N=2 matmuls);
    # emitted one output-tile group at a time, interleaved into phase 2's
    # first chunks so they never head-of-line block the projections ----
    #   qb[o] = sum_c wq[o,c] b[c] + bq    (added at the q PSUM->SBUF move)
    #   kb[o] = likewise with bk
    #   vb8[c] = sum_cin wv[c,cin] b[cin] + bv   (rides softmax into yb)
    #   yb[o] = sum_c wp[o,c] vb8[c] + bp        (y epilogue constant)
    vb8 = [ppool.tile([P, 2, 16], FP8, tag=f"vb8{t}", name=f"vb8{t}")
           for t in range(PT)]

    def bias_ct(wname, ot, outdt, addcol, tagp):
        pb = qpool.tile([P, 2], F32, tag="pa", name="pb")
        for t in range(CT):
            nc.tensor.matmul(pb[:], wts[wname][t][:, ot * P:(ot + 1) * P],
                             bvec[t][:, 0:2], start=(t == 0),
                             stop=(t == CT - 1))
        if outdt == F32:
            ob = ppool.tile([P, 1], F32, tag=f"{tagp}{ot}", name=f"{tagp}{ot}")
            nc.vector.tensor_scalar(ob[:], pb[:, 0:1], chvcol(ot, addcol),
                                    None, OP.add)
            return ob
        # fp8 pair column for the yb contract
        tf = smpool.tile([P, 1], F32, tag="tf", bufs=2)
        nc.vector.tensor_scalar(tf[:], pb[:, 0:1], chvcol(ot, addcol),
                                None, OP.add)
        nc.vector.tensor_copy(vb8[ot // 2][:, ot % 2, 0:1], tf[:])
        nc.vector.tensor_copy(vb8[ot // 2][:, ot % 2, 1:2], tf[:])
        return None

    def yb_ct(ot):
        pb = qpool.tile([P, 2], F32, tag="pa", name="pb")
        for t in range(PT):
            nc.tensor.matmul(pb[:], wp8[t][:, :, ot * P:(ot + 1) * P],
                             vb8[t][:, :, 0:2], start=(t == 0),
                             stop=(t == PT - 1), perf_mode=DR)
        ob = ppool.tile([P, 1], F32, tag=f"yb{ot}", name=f"yb{ot}")
        nc.vector.tensor_scalar(ob[:], pb[:, 0:1], chvcol(ot, 5),
                                None, OP.add)
        return ob

    # ---- persistent attention operands (all fp8 pairs) ----
    k8 = [ppool.tile([P, 2, HW], FP8, tag=f"k8{t}", name=f"k8{t}")
          for t in range(PT)]
    q8 = [ppool.tile([P, 2, QN], FP8, tag=f"q8{t}", name=f"q8{t}")
          for t in range(PT)]
    vT8 = [ppool.tile([P, 2, C], FP8, tag=f"vT8{j}", name=f"vT8{j}")
           for j in range(JJ)]

    # ---- phase 2: q/k/v projections straight from resident fp8 x; the
    # bias contracts ride along inside chunks 0-1 ----
    kb, qb, yb = [None] * CT, [None] * CT, [None] * CT
    p2tags = ["pa", "po0", "po1", "po2", "po3"]
    p2cnt = [0]

    def p2psum():
        tag = p2tags[p2cnt[0] % len(p2tags)]
        p2cnt[0] += 1
        return qpool.tile([P, CHW], F32, tag=tag,
                          bufs=(3 if tag == "pa" else 1), name="p2")

    for ch in range(HW // CHW):
        sl = slice(ch * CHW, (ch + 1) * CHW)
        for ot in range(CT):
            pk = p2psum()
            for t in range(PT):
                nc.tensor.matmul(pk[:], ws8["wkT"][t][:, :, ot * P:(ot + 1) * P],
                                 x8[t][:, :, sl], start=(t == 0),
                                 stop=(t == PT - 1), perf_mode=DR)
            if ch == 0:
                kb[ot] = bias_ct("wkT", ot, F32, 3, "kb")
            nc.vector.tensor_scalar(k8[ot // 2][:, ot % 2, sl], pk[:],
                                    kb[ot][:, 0:1], None, OP.add)
        for nt in range(CT):
            jt = ch * CT + nt
            pv = p2psum()
            for t in range(PT):
                nc.tensor.matmul(pv[:], x8[t][:, :, ch * CHW + nt * P:
                                              ch * CHW + (nt + 1) * P],
                                 ws8["wvT"][t][:], start=(t == 0),
                                 stop=(t == PT - 1), perf_mode=DR)
            if ch == 0:
                bias_ct("wvT", nt, FP8, 4, "vbt")
            elif ch == 1:
                yb[nt] = yb_ct(nt)
            nc.scalar.copy(vT8[jt // 2][:, jt % 2, :], pv[:])
        if ch * CHW < QN:
            for ot in range(CT):
                pq = p2psum()
                for t in range(PT):
                    nc.tensor.matmul(pq[:],
                                     ws8["wqT"][t][:, :, ot * P:(ot + 1) * P],
                                     x8[t][:, :, sl], start=(t == 0),
                                     stop=(t == PT - 1), perf_mode=DR)
                if ch == 0:
                    qb[ot] = bias_ct("wqT", ot, F32, 2, "qb")
                nc.vector.tensor_scalar(q8[ot // 2][:, ot % 2, sl], pq[:],
                                        qb[ot][:, 0:1], None, OP.add)

    # x + yb, precomputed off the critical path for the y epilogue
    xyb = [[None] * CT for _ in range(2)]
    for ih in range(2):
        for ot in range(CT):
            xt = ppool.tile([P, CHW], F32, tag=f"xyb{ih}{ot}",
                            name=f"xyb{ih}{ot}")
            nc.vector.tensor_scalar(xt[:],
                                    xb_sb[ot][:, ih * CHW:(ih + 1) * CHW],
                                    yb[ot][:, 0:1], None, OP.add)
            xyb[ih][ot] = xt

    # ---- phase 3: attention, per query half ----
    def mk_pr():
        return qpool.tile([1, CHW], F32, tag="pr", bufs=1, name="pr")

    def mk_po():
        return [qpool.tile([P, CHW], F32, tag=f"po{t}", name=f"po{t}", bufs=1)
                for t in range(CT)]

    def sc_exp(ih, j):
        isl = slice(ih * CHW, (ih + 1) * CHW)
        ps_ = qpool.tile([P, CHW], F32, tag="pa", name="ps")
        for t in range(PT):
            nc.tensor.matmul(ps_[:], k8[t][:, :, j * P:(j + 1) * P],
                             q8[t][:, :, isl], start=(t == 0),
                             stop=(t == PT - 1), perf_mode=DR)
        pT = spool.tile([P, 2, CHW], FP8, tag="pT", bufs=12, name="pT") \
            if j % 2 == 0 else None
        return ps_, pT

    def exp_into(pair, par, ps_):
        nc.scalar.activation(pair[:, par, :], ps_[:], AF.Exp, scale=SCALE,
                             bias=ebias[:, 0:1])

    def av_only(po, jj, pair):
        for t in range(CT):
            nc.tensor.matmul(po[t][:], vT8[jj][:, :, t * P:(t + 1) * P],
                             pair[:], start=(jj == 0), stop=(jj == JJ - 1),
                             perf_mode=DR)

    def rowsum_only(pr, jj, pair):
        nc.tensor.matmul(pr[:], ones8[:, :, 0:1], pair[:],
                         start=(jj == 0), stop=(jj == JJ - 1), perf_mode=DR)

    def tail_and_y(pr, po, ih):
        # 1/r chain first: pr closed early (rowsum bursts), so DVE computes
        # rinv/rbb and the broadcast matmul lands BEFORE the last AV
        # finishes; the softmax normalization folds into the fp8 'has'
        # tiles (has = po * (1/r)), so the epilogue is a single add
        rinv = smpool.tile([1, CHW], F32, tag="rinv", bufs=2)
        nc.vector.reciprocal_approx_fast(rinv[:], pr[:])
        rbb = smpool.tile([1, CHW], BF16, tag="rbb", bufs=2)
        nc.vector.tensor_copy(rbb[:], rinv[:])
        rb = spool.tile([P, CHW], F32, tag="rb", bufs=2)
        prb = qpool.tile([P, CHW], F32, tag="pa")
        nc.tensor.matmul(prb[:], ones_row[:], rbb[:], start=True, stop=True)
        nc.vector.tensor_copy(rb[:], prb[:])
        has = []
        for t in range(PT):
            ha = spool.tile([P, 2, CHW], FP8, tag=f"hx{t}", bufs=2)
            nc.vector.tensor_tensor(ha[:, 0, :], po[2 * t][:], rb[:],
                                    op=OP.mult)
            nc.vector.tensor_tensor(ha[:, 1, :], po[2 * t + 1][:], rb[:],
                                    op=OP.mult)
            has.append(ha)
        for ot in range(CT):
            # reuse the freed po slot: the pa slots stay available for the
            # next half's score pipeline even while the 1/r chain lags
            py = qpool.tile([P, CHW], F32, tag=f"po{ot}", name="py", bufs=1)
            for t in range(PT):
                nc.tensor.matmul(py[:], wp8[t][:, :, ot * P:(ot + 1) * P],
                                 has[t][:], start=(t == 0),
                                 stop=(t == PT - 1), perf_mode=DR)
            # column-split epilogue: each half fires its y DMA as soon as
            # its DVE add is done, hiding the per-transfer DMA latency
            yt = spool.tile([P, CHW], F32, tag="yt", bufs=3)
            for hc in range(2):
                cs = slice(hc * (CHW // 2), (hc + 1) * (CHW // 2))
                nc.vector.tensor_tensor(yt[:, cs], py[:, cs],
                                        xyb[ih][ot][:, cs], op=OP.add)
                (nc.gpsimd if (2 * ot + hc) % 2 == 0 else nc.sync).dma_start(
                    y_d[ot, :, ih * CHW + hc * (CHW // 2):
                        ih * CHW + (hc + 1) * (CHW // 2)], yt[:, cs])

    # scores/exp run LA jj-pairs ahead of the AV that consumes the pair:
    # by the time the PE reaches each consumer, the exp's (late-posting)
    # completion semaphore is stale and the PE never waits on ScalarE. The
    # next half's first KPRE pair groups are emitted into the drain/tail
    # window so the PE never idles across halves.
    LA, KPRE = 2, 3
    pr0 = mk_pr()
    po0 = mk_po()
    pr1 = mk_pr()
    pairs0, pre = {}, {}
    for it in range(JJ + KPRE):
        if it < JJ:
            ps0, pair = sc_exp(0, 2 * it)
            exp_into(pair, 0, ps0)
            ps1, _ = sc_exp(0, 2 * it + 1)
            exp_into(pair, 1, ps1)
            pairs0[it] = pair
        elif it - JJ < KPRE:
            jjp = it - JJ
            ps0, pair = sc_exp(1, 2 * jjp)
            exp_into(pair, 0, ps0)
            ps1, _ = sc_exp(1, 2 * jjp + 1)
            exp_into(pair, 1, ps1)
            pre[jjp] = pair
        # rowsums in bursts of 8 pairs: the M=1 matmul costs PE reconfig on
        # each entry/exit, so amortize it; the last burst lands before the
        # last AV so the 1/r chain overlaps it
        if it in (JJ // 2, JJ):
            for jp in range(it - JJ // 2, it):
                rowsum_only(pr0, jp, pairs0[jp])
        ja = it - LA
        if 0 <= ja < JJ:
            av_only(po0, ja, pairs0[ja])
    tail_and_y(pr0, po0, 0)
    po1 = mk_po()
    for it in range(JJ):
        js = it + KPRE
        if js < JJ:
            ps0, pair = sc_exp(1, 2 * js)
            exp_into(pair, 0, ps0)
            ps1, _ = sc_exp(1, 2 * js + 1)
            exp_into(pair, 1, ps1)
            pre[js] = pair
        if it in (JJ // 2 - 1, JJ - 1):
            for jp in range(it - JJ // 2 + 1, it + 1):
                rowsum_only(pr1, jp, pre[jp])
        av_only(po1, it, pre[it])
    tail_and_y(pr1, po1, 1)


def build_module():
    nc = bacc.Bacc("TRN2", target_bir_lowering=False, debug=False,
                   num_devices=NCORES)
    d = {
        "x8": nc.dram_tensor("x8", [PT, 4, P, 2, QN], FP8,
                             kind="ExternalInput").ap(),
        "xb": nc.dram_tensor("xb", [CT, 4, P, QN], BF16,
                             kind="ExternalInput").ap(),
        "wqT": nc.dram_tensor("wqT", [CT, P, C], BF16, kind="ExternalInput").ap(),
        "wkT": nc.dram_tensor("wkT", [CT, P, C], BF16, kind="ExternalInput").ap(),
        "wvT": nc.dram_tensor("wvT", [CT, P, C], BF16, kind="ExternalInput").ap(),
        "wp8": nc.dram_tensor("wp8", [PT, P, 2, C], FP8,
                              kind="ExternalInput").ap(),
        "chv": nc.dram_tensor("chv", [P, 6 * CT], F32, kind="ExternalInput").ap(),
        "ind8": nc.dram_tensor("ind8", [P, 2, 16], FP8,
                               kind="ExternalInput").ap(),
        "indTA": nc.dram_tensor("indTA", [16, P], F32,
                                kind="ExternalInput").ap(),
        "indTB": nc.dram_tensor("indTB", [16, P], F32,
                                kind="ExternalInput").ap(),
        "y": nc.dram_tensor("y", [CT, P, QN], F32, kind="ExternalOutput").ap(),
    }
    with tile.TileContext(nc) as tc, ExitStack() as ctx:
        _build_body(nc, tc, ctx, d)
    nc.compile()
    return nc


_CACHE = {}


def _get_nc():
    if "nc" not in _CACHE:
        _CACHE["nc"] = build_module()
    return _CACHE["nc"]


def _shared_inputs(gamma, beta, wq, bq, wk, bk, wv, bv, wp, bp):
    import ml_dtypes

    def wT(w):
        wt = np.ascontiguousarray(np.asarray(w, np.float32).T)
        return wt.reshape(CT, P, C).astype(ml_dtypes.bfloat16)

    wpT = np.asarray(wp, np.float32).T.reshape(PT, 2, P, C)
    wp8 = np.ascontiguousarray(wpT.transpose(0, 2, 1, 3)).astype(
        ml_dtypes.float8_e4m3)

    # group indicator for the PE stats matmuls: partition p / parity e of a
    # pair-tile belongs to group row 8*e + p//16
    ind8 = np.zeros((P, 2, 16), np.float32)
    for p in range(P):
        for e in range(2):
            ind8[p, e, 8 * e + p // 16] = 1.0
    # broadcast-back indicators (per parity): group row -> partition
    indTA = np.zeros((16, P), np.float32)
    indTB = np.zeros((16, P), np.float32)
    for p in range(P):
        indTA[p // 16, p] = 1.0
        indTB[8 + p // 16, p] = 1.0
    chv = np.stack([np.asarray(a, np.float32)
                    for a in (gamma, beta, bq, bk, bv, bp)],
                   axis=1).reshape(CT, P, 6).transpose(1, 0, 2).reshape(P, 24)
    return {
        "wqT": wT(wq), "wkT": wT(wk), "wvT": wT(wv), "wp8": wp8,
        "chv": np.ascontiguousarray(chv),
        "ind8": ind8.astype(ml_dtypes.float8_e4m3),
        "indTA": indTA, "indTB": indTB,
    }


def make_in_maps(x, gamma, beta, wq, bq, wk, bk, wv, bv, wp, bp):
    import ml_dtypes

    shared = _shared_inputs(gamma, beta, wq, bq, wk, bk, wv, bv, wp, bp)
    xf = np.asarray(x, np.float32).reshape(B, C, HW)
    in_maps = []
    for core in range(NCORES):
        b, qc = divmod(core, NCORES // B)
        xr = np.roll(xf[b], -qc * QN, axis=1)          # [C, HW]
        # fp8 channel-pair layout [T, qtr, p, e, m]: c = 256T+128e+p
        x8 = xr.reshape(PT, 2, P, 4, QN).transpose(0, 3, 2, 1, 4)
        xt = xr.reshape(CT, P, 4, QN).transpose(0, 2, 1, 3)
        m = dict(shared)
        m["x8"] = np.ascontiguousarray(x8).astype(ml_dtypes.float8_e4m3)
        m["xb"] = np.ascontiguousarray(xt).astype(ml_dtypes.bfloat16)
        in_maps.append(m)
    return in_maps


def assemble_output(results):
    out = np.empty((B, C, HW), np.float32)
    for core in range(NCORES):
        b, qc = divmod(core, NCORES // B)
        y = np.asarray(results[core]["y"]).reshape(C, QN)
        out[b, :, qc * QN:(qc + 1) * QN] = y
    return out.reshape(B, C, H, W)


def kernel(x, gamma, beta, wq, bq, wk, bk, wv, bv, wp, bp):
    nc = _get_nc()
    in_maps = make_in_maps(x, gamma, beta, wq, bq, wk, bk, wv, bv, wp, bp)
    res = run_bass_kernel_spmd(nc, in_maps, list(range(NCORES)))
    return assemble_output(res.results)
